# revision 19
# baseline (speedup 1.0000x reference)
"""Trainium2 Bass kernel for a single-head causal attention block.

Problem (hardcoded):
  input_val: [4, 4096, 1024] f32, Wq/Wk/Wv: [64, 1024] f32, k_mask: [4, 4096] i32
  out = softmax(causal_mask(QK^T/sqrt(64))) @ V  -> [4, 4096, 64] f32

Sharding: 8 cores = 4 batches x 2 roles. Within a batch, queries are split
into 16 chunks of 256; role r takes global chunks {2j+r}. Every core runs an
IDENTICAL program (SPMD); the role is carried entirely by the per-core input
data: the host swaps the two 256-key halves of every 512-key t-block for
role 1, so that each core's own query rows always sit at columns 0:256 of
its t-blocks, and rebuilds the causal masks to match the permuted key order.

Device program per core:
  - X is loaded once as X^T t-blocks (xt, bf16); all 8 t-block DMAs are
    issued up front (tb0 split in half) so the PE never starves.
  - K^T/V^T projection per t-block (bf16, contraction chunked over C)
    -> PSUM [K^T 0:64 | V^T 64:128, 512]. DVE evacuates K^T to KTF[0:64]
    and duplicates it to KTF[64:128] (feeds the second PE row-tile);
    V^T goes (partition-shifted) to VT0[0:64].
  - V1[kb] = [V | ones] built by a hardware DMA transpose per t-block
    (VT0 -> V1, bf16). ones column memset once.
  - Q^T projection reads X^T straight from xt (cols 0:256 of its two
    t-blocks, strided rhs) in 8 N=512 matmuls; DVE copies + duplicates.
  - attention in S^T layout over chunk-pairs m (512 queries), flash style:
    shared kb pairs run as two CONCURRENT row-tiled K=64 matmuls
    (tile_position (0,0)/(64,0), separate PSUM banks), one ACT exp
    (N=1024), DVE causal mask on diagonal/tail slots, then per kb one
    O^T matmul [65,512] += V1[kb].T @ P^T (65th col = ones gives the
    softmax denominator for free).
  - out [65, 2048] f32; host divides rows 0:64 by row 64 and scatters.
"""
import numpy as np

B, T, C, H = 4, 4096, 1024, 64
N_CORES = 8
CC = 8          # contraction chunks of 128 over C
TB = 8          # key t-blocks of 512
NKB = 32        # key blocks of 128
NPAIR = 4       # local chunk-pairs of 512 queries
SCALE = 1.0 / np.sqrt(H)

_CACHE = {}


def _build_program(use_kmask: bool):
    from contextlib import ExitStack
    import concourse.tile as tile
    from concourse import bacc, mybir

    BF16 = mybir.dt.bfloat16
    F32 = mybir.dt.float32
    FP8 = mybir.dt.float8e4
    Exp = mybir.ActivationFunctionType.Exp
    DR = mybir.MatmulPerfMode.DoubleRow

    nc = bacc.Bacc("TRN2", target_bir_lowering=False, debug=False)
    xt = nc.dram_tensor("xt", [TB, 128, CC, 512], BF16, kind="ExternalInput")
    wkv = nc.dram_tensor("wkv", [128, CC, 128], BF16, kind="ExternalInput")
    wq = nc.dram_tensor("wq", [128, CC, 64], BF16, kind="ExternalInput")
    cm2 = nc.dram_tensor("cm2", [128, 4, 512], BF16, kind="ExternalInput")
    cm1 = nc.dram_tensor("cm1", [128, 2, 512], BF16, kind="ExternalInput")
    if use_kmask:
        km = nc.dram_tensor("km", [128, NKB], F32, kind="ExternalInput")
    o = nc.dram_tensor("o", [65, 2048], F32, kind="ExternalOutput")

    with tile.TileContext(nc) as tc:
        with ExitStack() as ctx:
            const = ctx.enter_context(tc.tile_pool(name="const", bufs=1))
            ppool = ctx.enter_context(tc.tile_pool(name="ptp", bufs=4))
            projp = ctx.enter_context(tc.tile_pool(name="projp", bufs=2, space="PSUM"))
            stp = ctx.enter_context(tc.tile_pool(name="stp", bufs=2, space="PSUM"))
            otp = ctx.enter_context(tc.tile_pool(name="otp", bufs=2, space="PSUM"))

            XT = const.tile([128, TB, CC, 512], BF16)
            WKV = const.tile([128, CC, 128], BF16)
            WQ = const.tile([128, CC, 64], BF16)
            CM2 = const.tile([128, 4, 512], BF16)
            CM1 = const.tile([128, 2, 512], BF16)
            KTF = const.tile([128, T], BF16)
            QTF = const.tile([128, 2048], BF16)
            VT0 = const.tile([64, T], BF16)
            V1 = const.tile([128, NKB, 128], BF16)  # [:, kb, 0:64]=V, col 64=1, rest 0
            OUT = const.tile([65, 2048], F32)
            WARM = const.tile([128, 128], BF16)
            if use_kmask:
                KM = const.tile([128, NKB], F32)

            # gpsimd queue: memsets FIRST (warmup + ones cols), then all xt
            # t-block DMAs up front (tb0 split so matmuls start earlier).
            nc.gpsimd.memset(WARM[:], 0.25)
            # xt rides the gpsimd SWDGE ring only (the sync HWDGE ring is
            # several times slower for bulk transfers), staggered so at most
            # ~4 DMAs are outstanding: three up front, the rest issued
            # just-in-time from the pair infra slots below.
            nc.gpsimd.dma_start(XT[:, 0, 0:4], xt.ap()[0][:, 0:4])
            nc.gpsimd.dma_start(XT[:, 0, 4:8], xt.ap()[0][:, 4:8])
            nc.gpsimd.dma_start(XT[:, 1], xt.ap()[1])
            nc.gpsimd.memset(V1[:, :, 64:128], 0.0)
            nc.gpsimd.memset(V1[:, :, 64:65], 1.0)
            nc.sync.dma_start(WKV[:], wkv.ap())
            nc.sync.dma_start(WQ[:], wq.ap())
            nc.sync.dma_start(CM2[:], cm2.ap())
            nc.sync.dma_start(CM1[:], cm1.ap())
            if use_kmask:
                nc.sync.dma_start(KM[:], km.ap())

            # HAM warm-up: dependency-free matmuls on memset scratch keep the
            # PE busy until the first input DMA lands so every real matmul
            # runs at 2.4GHz.
            wps = projp.tile([128, 512], F32, name="wps", tag="ps")
            for _ in range(55):
                nc.tensor.matmul(wps[:, 0:128], WARM[:], WARM[:],
                                 start=True, stop=True)

            def emit_kv(tb):
                ps = projp.tile([128, 512], F32, name="ps", tag="ps")
                for cc in range(CC):
                    nc.tensor.matmul(ps[:], WKV[:, cc, :], XT[:, tb, cc, :],
                                     start=(cc == 0), stop=(cc == CC - 1))
                sl = slice(512 * tb, 512 * (tb + 1))
                nc.vector.tensor_copy(KTF[0:64, sl], ps[0:64, :])
                nc.vector.tensor_copy(VT0[0:64, sl], ps[64:128, :])
                nc.vector.tensor_copy(KTF[64:128, sl], KTF[0:64, sl])

            def emit_trans(tb):
                nc.sync.dma_start_transpose(V1[:, 4 * tb:4 * tb + 4, 0:64],
                                            VT0[:, 512 * tb:512 * (tb + 1)])

            def xdma(tb):
                if tb < TB:
                    nc.gpsimd.dma_start(XT[:, tb], xt.ap()[tb])

            def emit_q(m):
                ps = projp.tile([128, 512], F32, name="ps", tag="ps")
                for cc in range(CC):
                    nc.tensor.matmul(ps[0:64, :], WQ[:, cc, :],
                                     XT[:, 2 * m:2 * m + 2, cc, 0:256],
                                     start=(cc == 0), stop=(cc == CC - 1))
                qsl = slice(512 * m, 512 * (m + 1))
                nc.vector.tensor_copy(QTF[0:64, qsl], ps[0:64, :])
                nc.vector.tensor_copy(QTF[64:128, qsl], QTF[0:64, qsl])

            def kmul(pt_slice, kb):
                nc.vector.tensor_scalar_mul(pt_slice, pt_slice, KM[:, kb:kb + 1])

            # Schraudolph exp for ACT offload: exp(st/8) ~ bf16 whose bits
            # are round(st*EXA + EXB); ~1.5% per-element err, used only on a
            # few early mask-free slots of the ACT-bound last pair.
            EXA = float(SCALE * 128.0 / np.log(2.0))
            EXB = 16256.0 - 8.5

            def emit_S_shared(m, sp, approx=False):
                st = stp.tile([128, 2, 512], F32, name="st", tag="st")
                qsl = slice(512 * m, 512 * (m + 1))
                kb0, kb1 = 2 * sp, 2 * sp + 1
                nc.tensor.matmul(st[:, 0, :], KTF[0:64, 128 * kb0:128 * (kb0 + 1)],
                                 QTF[0:64, qsl], start=True, stop=True,
                                 tile_position=(0, 0))
                nc.tensor.matmul(st[:, 1, :], KTF[64:128, 128 * kb1:128 * (kb1 + 1)],
                                 QTF[64:128, qsl], start=True, stop=True,
                                 tile_position=(64, 0))
                pt = ppool.tile([128, 2, 512], BF16, name="pt", tag="pt")
                if approx:
                    nc.vector.tensor_scalar(pt[:].bitcast(mybir.dt.int16), st[:],
                                            EXA, EXB, mybir.AluOpType.mult,
                                            mybir.AluOpType.add)
                else:
                    nc.scalar.activation(pt[:], st[:], Exp, scale=SCALE)
                if sp >= 4 * m:
                    t0 = 2 * (sp - 4 * m)
                    nc.vector.tensor_mul(pt[:], pt[:], CM2[:, t0:t0 + 2, :])
                if use_kmask:
                    for u in range(2):
                        kmul(pt[:, u, :], 2 * sp + u)
                return pt

            def emit_O_shared(m, sp, pt, oacc):
                for u in range(2):
                    kb = 2 * sp + u
                    nc.tensor.matmul(oacc[:], V1[:, kb, :], pt[:, u, :],
                                     start=(kb == 0), stop=False)

            def emit_S_tail(m):
                # tail kb t (=0..3) -> tile u=t%2, col half v=t//2
                st = stp.tile([128, 2, 512], F32, name="st", tag="st")
                qsl = slice(512 * m + 256, 512 * (m + 1))
                for t in range(4):
                    kb = 8 * m + 4 + t
                    u, v = t % 2, t // 2
                    nc.tensor.matmul(
                        st[:, u, 256 * v:256 * v + 256],
                        KTF[64 * u:64 * u + 64, 128 * kb:128 * (kb + 1)],
                        QTF[64 * u:64 * u + 64, qsl], start=True, stop=True,
                        tile_position=(64 * u, 0))
                pt = ppool.tile([128, 2, 512], BF16, name="pt", tag="pt")
                nc.scalar.activation(pt[:], st[:], Exp, scale=SCALE)
                nc.vector.tensor_mul(pt[:], pt[:], CM1[:])
                if use_kmask:
                    for t in range(4):
                        kmul(pt[:, t % 2, 256 * (t // 2):256 * (t // 2) + 256],
                             8 * m + 4 + t)
                return pt

            def emit_O_tail(m, pt, oacc):
                for t in range(4):
                    kb = 8 * m + 4 + t
                    u, v = t % 2, t // 2
                    nc.tensor.matmul(
                        oacc[:, 256:512], V1[:, kb, :],
                        pt[:, u, 256 * v:256 * v + 256],
                        start=False, stop=(t == 3))
                nc.vector.tensor_copy(OUT[:, 512 * m:512 * (m + 1)], oacc[0:65, :])
                nc.sync.dma_start(o.ap()[:, 512 * m:512 * (m + 1)],
                                  OUT[:, 512 * m:512 * (m + 1)])

            # skewed emission: the O-matmuls of work item i are emitted two
            # S-slots later, keeping PE fed during exp latency.
            pending = []  # [(kind, m, sp, pt), ...]
            oaccs = {}

            def flush_one():
                kind, m_, sp_, pt_ = pending.pop(0)
                if kind == "shared":
                    emit_O_shared(m_, sp_, pt_, oaccs[m_])
                else:
                    emit_O_tail(m_, pt_, oaccs[m_])

            def flush_pending(depth=2):
                while len(pending) > depth:
                    flush_one()

            # Per pair m, interleave infra (projections, V transposes) with
            # the attention slots so the PE always has queued work while
            # ACT/DVE drain the S->P chain.
            for m in range(NPAIR):
                oaccs[m] = otp.tile([128, 512], F32, name="oacc", tag="oacc")
                infra = [lambda m=m: emit_kv(2 * m),
                         lambda m=m: (emit_q(m), xdma(2 * m + 2)),
                         None,  # S(0) goes here
                         lambda m=m: (emit_trans(2 * m), xdma(2 * m + 3)),
                         None,  # S(1) goes here
                         lambda m=m: emit_kv(2 * m + 1),
                         None,  # S(2)
                         lambda m=m: emit_trans(2 * m + 1)]
                infra[0]()
                if pending:
                    flush_one()
                infra[1]()
                if pending:
                    flush_one()
                sp = 0
                nslots = 4 * m + 2
                apx = (lambda s: m == NPAIR - 1 and s < 3 and not use_kmask)
                pt = emit_S_shared(m, 0, apx(0))
                pending.append(("shared", m, 0, pt))
                flush_pending()
                for step in (3, 5, 7):
                    infra[step]()
                    if sp + 1 < nslots:
                        sp += 1
                        pt = emit_S_shared(m, sp, apx(sp))
                        pending.append(("shared", m, sp, pt))
                        flush_pending()
                while sp + 1 < nslots:
                    sp += 1
                    pt = emit_S_shared(m, sp, apx(sp))
                    pending.append(("shared", m, sp, pt))
                    flush_pending()
                pt = emit_S_tail(m)
                pending.append(("tail", m, None, pt))
                flush_pending()
            flush_pending(depth=0)

    nc.compile()
    return nc


def _get_program(use_kmask: bool):
    key = ("prog", use_kmask)
    if key not in _CACHE:
        _CACHE[key] = _build_program(use_kmask)
    return _CACHE[key]


def _host_prep(input_val, Wq, Wk, Wv, k_mask, use_kmask):
    import ml_dtypes
    bf = ml_dtypes.bfloat16
    f8 = ml_dtypes.float8_e4m3fn

    wkv_np = np.empty((128, CC, 128), dtype=bf)
    wkv_np[:, :, 0:64] = Wk.reshape(64, CC, 128).transpose(2, 1, 0).astype(bf)
    wkv_np[:, :, 64:128] = Wv.reshape(64, CC, 128).transpose(2, 1, 0).astype(bf)
    wq_np = Wq.reshape(64, CC, 128).transpose(2, 1, 0).astype(bf).copy()

    # key-column permutation per role: role 1 swaps the 256-halves of each
    # 512-key t-block (so each core's own queries sit at cols 0:256).
    kgrel = {0: np.arange(512), 1: (np.arange(512) + 256) % 512}

    cm2s, cm1s = [], []
    for r in range(2):
        kg = kgrel[r]
        kk = np.arange(128)
        qq = np.arange(256)
        # shared diagonal masks: slot kbs t=0..3 cover one t-block; queries:
        # col j: qrel = 512*(j//256) + 256*r + j%256
        c2 = np.ones((128, 4, 512), dtype=np.float32)
        for t in range(4):
            keyg = kg[128 * t + kk]
            c2[:, t, 0:256] = (keyg[:, None] <= (256 * r + qq)[None, :])
        cm2s.append(c2.astype(bf))
        # tail masks: kb t -> layout [u=t%2, 256*(t//2)+qq]; queries are the
        # upper chunk only: qrel = 256*r + qq (keys are in the NEXT t-block)
        c1 = np.empty((128, 2, 512), dtype=np.float32)
        for t in range(4):
            keyg = kg[128 * t + kk]
            u, v = t % 2, t // 2
            c1[:, u, 256 * v:256 * v + 256] = (keyg[:, None] <= (256 * r + qq)[None, :])
        cm1s.append(c1.astype(bf))

    # xt per (batch, role): [TB, 128, CC, 512] bf16, with role-1 key swap
    xts = {}
    for b in range(B):
        Xb = np.asarray(input_val[b], dtype=np.float32)
        for r in range(2):
            Xr = Xb.reshape(TB, 2, 256, C)
            if r == 1:
                Xr = Xr[:, ::-1]
            xts[(b, r)] = np.ascontiguousarray(
                Xr.reshape(TB, 512, CC, 128).transpose(0, 3, 2, 1)).astype(bf)

    in_maps = []
    for c in range(N_CORES):
        b, r = c // 2, c % 2
        m = {"xt": xts[(b, r)], "wkv": wkv_np, "wq": wq_np,
             "cm2": cm2s[r], "cm1": cm1s[r]}
        if use_kmask:
            kmb = np.asarray(k_mask[b], dtype=np.float32)
            kmr = kmb.reshape(TB, 512)[:, kgrel[r]].reshape(NKB, 128)
            m["km"] = np.ascontiguousarray(kmr.T)
        in_maps.append(m)
    return in_maps


def _unshard(results):
    out = np.empty((B, T, H), dtype=np.float32)
    for c in range(N_CORES):
        b, r = c // 2, c % 2
        on = results[c]["o"]
        num = on[0:64, :]
        den = on[64, :]
        for j in range(2 * NPAIR):
            g = 2 * j + r
            blk = num[:, 256 * j:256 * (j + 1)] / den[None, 256 * j:256 * (j + 1)]
            out[b, 256 * g:256 * (g + 1), :] = blk.T
    return out


def kernel(input_val, Wq, Wk, Wv, k_mask):
    import concourse.bass_utils as bu

    input_val = np.asarray(input_val)
    Wq, Wk, Wv = (np.asarray(a, dtype=np.float32) for a in (Wq, Wk, Wv))
    k_mask = np.asarray(k_mask)
    use_kmask = not bool(np.all(k_mask == 1))

    nc = _get_program(use_kmask)
    in_maps = _host_prep(input_val, Wq, Wk, Wv, k_mask, use_kmask)
    res = bu.run_bass_kernel_spmd(nc, in_maps, core_ids=list(range(N_CORES)))
    return _unshard(res.results)


def kernel_traced(input_val, Wq, Wk, Wv, k_mask, **trace_kwargs):
    """Like kernel() but returns (output, BassKernelResults) with tracing on."""
    import concourse.bass_utils as bu

    input_val = np.asarray(input_val)
    k_mask = np.asarray(k_mask)
    use_kmask = not bool(np.all(k_mask == 1))
    nc = _get_program(use_kmask)
    in_maps = _host_prep(input_val, np.asarray(Wq, dtype=np.float32),
                         np.asarray(Wk, dtype=np.float32),
                         np.asarray(Wv, dtype=np.float32), k_mask, use_kmask)
    res = bu.run_bass_kernel_spmd(nc, in_maps, core_ids=list(range(N_CORES)),
                                  trace=True, **trace_kwargs)
    return _unshard(res.results), res
